# revision 31
# baseline (speedup 1.0000x reference)
"""Trainium2 Bass kernel for a device-aware top-1 MoE layer.

Strategy (expert parallelism over 8 NeuronCores):
  - Host: gate + top-1 routing, then pack each expert's tokens.
    Experts are paired big+small across cores (sorted by count) so the
    program's two capacity slots (C1 >= C2) waste little padding.
  - Device (SPMD, one NEFF on 8 cores): core holds 2 experts in fp8
    e3m4 (weights pre-scaled by 2^8 / 2^9 so they sit in e3m4's normal
    range; power-of-two scales are exact).  Activations are fp16.
      stage 1: h' = relu(w1q.T @ xT + 2^8*b1)     (= 2^8 * h, exact)
      stage 2: y  = (w2q.T @ h') * 2^-17 + b2     (epilogue scale)
    fp32 PSUM accumulation throughout.
  - Host: scatter each expert's [D, count] output back to token rows.

Perf notes:
  - fp8 weights halve HBM traffic vs bf16 (8 MB/core weights) AND keep
    the PE fed: the PE consumes fp8 weights at ~250 GB/s < 358 GB/s DMA,
    so the tensor engine (not DMA) sets the pace after warmup.
  - Every DRAM tensor is host-packed into its exact SBUF image
    [128, bytes] so all DMA descriptors are long contiguous lines
    (2-16 KB), maximizing HBM efficiency.
  - All weight DMA rides the sync HWDGE queue in exact consumption
    order; stage 2 iterates h-outer so the PE consumes w2 tiles as they
    arrive.  Slot-0 output is written early on the gpsimd queue to
    overlap slot-1 compute.
"""

import numpy as np
import ml_dtypes

D = 1024
H = 2048
E = 16
NCORES = 8
P = 128
DB = D // P   # 8 d-chunks
HB = H // P   # 16 h-chunks
W1G = 2       # d-chunks per slot-1 w1 DMA (0.5 MB, 4 KB descriptors)
W2G = 4       # h-chunks per w2 DMA (0.5 MB, 4 KB descriptors)
S1 = 256.0    # 2^8  w1 scale
S2 = 512.0    # 2^9  w2 scale
UNSCALE = 1.0 / (S1 * S2)

_program_cache = {}


def _build_program(C1, C2):
    """Trace the per-core Bass/Tile program for capacities (C1, C2)."""
    import concourse.tile as tile
    from concourse import bacc, mybir

    assert C1 <= 512 and C2 <= C1
    f32 = mybir.dt.float32
    f16 = mybir.dt.float16
    f8 = mybir.dt.float8e3
    AF = mybir.ActivationFunctionType
    ALU = mybir.AluOpType
    CS = (C1, C2)

    nc = bacc.Bacc(
        "TRN2", target_bir_lowering=False, debug=False, num_devices=NCORES
    )
    xT = nc.dram_tensor("xT", [P, DB * (C1 + C2)], f16, kind="ExternalInput").ap()
    w1s = nc.dram_tensor("w1s", [2, P, DB * H], f8, kind="ExternalInput").ap()
    w2s = nc.dram_tensor("w2s", [2, P, HB * D], f8, kind="ExternalInput").ap()
    b1s = nc.dram_tensor("b1s", [2, P, HB], f32, kind="ExternalInput").ap()
    b2s = nc.dram_tensor("b2s", [2, P, DB], f32, kind="ExternalInput").ap()
    y0 = nc.dram_tensor("y0", [P, DB * C1], f16, kind="ExternalOutput").ap()
    y1 = nc.dram_tensor("y1", [P, DB * C2], f16, kind="ExternalOutput").ap()
    ys = (y0, y1)

    with tile.TileContext(nc) as tc:
        with (
            tc.tile_pool(name="xp", bufs=1) as xp,
            tc.tile_pool(name="w1p", bufs=8) as w1p,
            tc.tile_pool(name="w2p", bufs=8) as w2p,
            tc.tile_pool(name="hp", bufs=32) as hp,
            tc.tile_pool(name="bp", bufs=2) as bp,
            tc.tile_pool(name="yp", bufs=2) as yp,
            tc.tile_pool(name="ps", bufs=8, space="PSUM") as ps,
        ):
            xts = [None, None]
            hts = [[None] * HB for _ in range(2)]
            b1ts = [None, None]
            b2ts = [None, None]

            # Tiny bias tiles ride the gpsimd queue.
            for s in range(2):
                b1t = bp.tile([P, HB], f32, tag="b1")
                nc.gpsimd.dma_start(b1t[:], b1s[s])
                b1ts[s] = b1t
                b2t = bp.tile([P, DB], f32, tag="b2")
                nc.gpsimd.dma_start(b2t[:], b2s[s])
                b2ts[s] = b2t

            def epi1(i, out_t, acc_t, bias_col):
                """relu(acc + b1s) from PSUM to SBUF, alternating engines."""
                if i % 2 == 0:
                    nc.scalar.activation(out_t[:], acc_t[:], AF.Relu, bias=bias_col)
                else:
                    nc.vector.tensor_scalar(
                        out_t[:], acc_t[:], bias_col, 0.0, ALU.add, ALU.max
                    )

            def epi2(i, out_t, acc_t, bias_col):
                """acc * 2^-17 + b2 from PSUM to SBUF, alternating engines."""
                if i % 2 == 0:
                    nc.scalar.activation(
                        out_t[:], acc_t[:], AF.Identity,
                        bias=bias_col, scale=UNSCALE,
                    )
                else:
                    nc.vector.tensor_scalar(
                        out_t[:], acc_t[:], UNSCALE, bias_col, ALU.mult, ALU.add
                    )

            xoff = [0, DB * C1]

            # ---------------- slot 0 (big expert, C1) ----------------
            # Head-latency critical.  Two hardware DGEs exist (sync=SP
            # and scalar=Activation): x rides the scalar DGE while the
            # weight stream owns the sync DGE, so descriptor generation
            # for both runs in parallel and the first matmul's inputs
            # (x[d0] + w1[d0], ~0.3 MB) land as early as possible.
            C = C1
            # Everything on the sync DGE in exact consumption order: both
            # HWDGE queues share the same 16 physical DMA engines (no
            # extra bandwidth), so a single in-order stream keeps the
            # d-outer ramp starvation-free.
            xa = xp.tile([P, C], f16, tag="xa")
            nc.sync.dma_start(xa[:], xT[:, 0:C])
            w1ts0 = []
            w1t = w1p.tile([P, H], f8, tag="w1a")
            nc.sync.dma_start(w1t[:], w1s[0, :, 0:H])
            w1ts0.append((w1t[:], 0))
            xb = xp.tile([P, (DB - 1) * C], f16, tag="xb")
            nc.sync.dma_start(xb[:], xT[:, C:DB * C])
            for d in range(1, DB):
                w1t = w1p.tile([P, H], f8, tag="w1a")
                nc.sync.dma_start(w1t[:], w1s[0, :, d * H:(d + 1) * H])
                w1ts0.append((w1t[:], 0))

            def rhs0(d):
                return xa[:] if d == 0 else xb[:, (d - 1) * C:d * C]

            # stage 1: two half-H passes, d-outer within each, so the PE
            # starts on w1[d0] immediately (second pass reuses resident
            # w1 tiles).
            for half in range(2):
                accs1 = [
                    ps.tile([P, C], f32, tag="acc", name=f"acc1_{half}_{i}")
                    for i in range(HB // 2)
                ]
                for d in range(DB):
                    w1t, dd = w1ts0[d]
                    for hh in range(HB // 2):
                        h = half * (HB // 2) + hh
                        nc.tensor.matmul(
                            accs1[hh][:],
                            lhsT=w1t[:, dd * H + h * P:dd * H + h * P + P],
                            rhs=rhs0(d),
                            start=(d == 0),
                            stop=(d == DB - 1),
                        )
                for hh in range(HB // 2):
                    h = half * (HB // 2) + hh
                    ht = hp.tile([P, C], f16, tag="hT")
                    epi1(h, ht, accs1[hh], b1ts[0][:, h:h + 1])
                    hts[0][h] = ht

            # stage 2: h-outer so the PE consumes each w2 tile as soon
            # as its DMA lands.
            accs = [
                ps.tile([P, C], f32, tag="acc", name=f"acc2_0_{d}")
                for d in range(DB)
            ]
            for g in range(HB // W2G):
                w2t = w2p.tile([P, W2G * D], f8, tag="w2")
                nc.sync.dma_start(
                    w2t[:], w2s[0, :, g * W2G * D:(g + 1) * W2G * D]
                )
                for hh in range(W2G):
                    h = g * W2G + hh
                    for d in range(DB):
                        nc.tensor.matmul(
                            accs[d][:],
                            lhsT=w2t[:, hh * D + d * P:hh * D + d * P + P],
                            rhs=hts[0][h][:],
                            start=(h == 0),
                            stop=(h == HB - 1),
                        )
            yt0 = yp.tile([P, DB * C], f16, tag="yt")
            for d in range(DB):
                epi2(d, yt0[:, d * C:(d + 1) * C], accs[d],
                     b2ts[0][:, d:d + 1])
            # Slot 0's output rides the (otherwise idle) gpsimd queue so
            # it overlaps slot 1 compute without blocking weight loads.
            nc.gpsimd.dma_start(ys[0], yt0[:])

            # ---------------- slot 1 (small expert, C2) ----------------
            C = CS[1]
            xt1 = xp.tile([P, DB * C], f16, tag="xc")
            nc.sync.dma_start(xt1[:], xT[:, xoff[1]:xoff[1] + DB * C])
            xts[1] = xt1
            w1ts = []
            for g in range(DB // W1G):
                w1t = w1p.tile([P, W1G * H], f8, tag="w1b", bufs=4)
                nc.sync.dma_start(
                    w1t[:], w1s[1, :, g * W1G * H:(g + 1) * W1G * H]
                )
                w1ts.append(w1t)
            # All of slot 1's w2 too: it is fully resident long before
            # stage 2 below reaches it (PE is the bottleneck by then).
            w2ts = []
            for g in range(HB // W2G):
                w2t = w2p.tile([P, W2G * D], f8, tag="w2")
                nc.sync.dma_start(
                    w2t[:], w2s[1, :, g * W2G * D:(g + 1) * W2G * D]
                )
                w2ts.append(w2t)

            # stage 1: h-outer retires each psum right away so the
            # epilogues pipeline with the next chain's matmuls.
            for h in range(HB):
                acc = ps.tile([P, C], f32, tag="acc")
                for d in range(DB):
                    w1t = w1ts[d // W1G]
                    col = (d % W1G) * H + h * P
                    nc.tensor.matmul(
                        acc[:],
                        lhsT=w1t[:, col:col + P],
                        rhs=xts[1][:, d * C:(d + 1) * C],
                        start=(d == 0),
                        stop=(d == DB - 1),
                    )
                ht = hp.tile([P, C], f16, tag="hT")
                epi1(h, ht, acc, b1ts[1][:, h:h + 1])
                hts[1][h] = ht

            # stage 2, d-blocked: each d-chunk runs its full h-chain,
            # retires its epilogue, and streams out in 2-chunk writes
            # while later chains still compute -- the tail after the
            # last matmul is one epilogue + one small DMA.
            yt1 = yp.tile([P, DB * C], f16, tag="yt")
            for d in range(DB):
                acc = ps.tile([P, C], f32, tag="acc", name=f"acc2_1_{d}")
                for h in range(HB):
                    g, hh = h // W2G, h % W2G
                    nc.tensor.matmul(
                        acc[:],
                        lhsT=w2ts[g][:, hh * D + d * P:hh * D + d * P + P],
                        rhs=hts[1][h][:],
                        start=(h == 0),
                        stop=(h == HB - 1),
                    )
                epi2(d, yt1[:, d * C:(d + 1) * C], acc, b2ts[1][:, d:d + 1])
                if d % 2 == 1:
                    lo = (d - 1) * C
                    nc.sync.dma_start(
                        ys[1][:, lo:(d + 1) * C], yt1[:, lo:(d + 1) * C]
                    )

    nc.compile()
    return nc


def kernel(x, gate_w, gate_b, w1, b1, w2, b2, _trace=False):
    from concourse.bass_utils import run_bass_kernel_spmd

    f16 = np.float16
    e3m4 = ml_dtypes.float8_e3m4

    x = np.asarray(x, dtype=np.float32)
    B, S, d_in = x.shape
    T = B * S
    xf = x.reshape(T, d_in)

    # --- routing (host side: this is the dispatch/sharding step) ---
    logits = xf @ np.asarray(gate_w, dtype=np.float32) + np.asarray(
        gate_b, dtype=np.float32
    )
    top1 = np.argmax(logits, axis=-1)
    idxs = [np.nonzero(top1 == e)[0] for e in range(E)]
    counts = np.array([len(i) for i in idxs])

    # Pair big+small experts per core: slot 0 gets the 8 largest.
    order = np.argsort(-counts, kind="stable")
    slot_experts = [
        (int(order[core]), int(order[2 * NCORES - 1 - core]))
        for core in range(NCORES)
    ]

    def cap(n):
        return min(512, max(16, (n + 3) // 4 * 4))

    C1 = cap(max(counts[e0] for e0, _ in slot_experts))
    C2 = cap(max(counts[e1] for _, e1 in slot_experts))
    assert all(counts[a] <= C1 and counts[b] <= C2 for a, b in slot_experts)

    if (C1, C2) not in _program_cache:
        _program_cache[(C1, C2)] = _build_program(C1, C2)
    nc = _program_cache[(C1, C2)]

    # Pre-quantize all expert weights into their SBUF image layouts.
    w1q = (np.asarray(w1, dtype=np.float32) * S1).astype(e3m4)   # [E, D, H]
    w2q = (np.asarray(w2, dtype=np.float32) * S2).astype(e3m4)   # [E, H, D]
    b1f = np.asarray(b1, dtype=np.float32) * S1
    b2f = np.asarray(b2, dtype=np.float32)
    xf16 = xf.astype(f16)
    CS = (C1, C2)

    in_maps = []
    for core in range(NCORES):
        xT = np.zeros((P, DB * (C1 + C2)), dtype=f16)
        w1sv = np.empty((2, P, DB * H), dtype=e3m4)
        w2sv = np.empty((2, P, HB * D), dtype=e3m4)
        b1sv = np.empty((2, P, HB), dtype=np.float32)
        b2sv = np.empty((2, P, DB), dtype=np.float32)
        xoff = (0, DB * C1)
        for s in range(2):
            e = slot_experts[core][s]
            C = CS[s]
            idx = idxs[e]
            n = len(idx)
            if n:
                # [p, dd*C + c] = x[idx[c], dd*128 + p]
                xs = xf16[idx].T.reshape(DB, P, n).transpose(1, 0, 2)
                xv = xT[:, xoff[s]:xoff[s] + DB * C].reshape(P, DB, C)
                xv[:, :, :n] = xs
            # [p, dd*H + h] = w1q[e][dd*128 + p, h]
            w1sv[s] = (
                w1q[e].reshape(DB, P, H).transpose(1, 0, 2).reshape(P, DB * H)
            )
            # [p, oo*D + d] = w2q[e][oo*128 + p, d]
            w2sv[s] = (
                w2q[e].reshape(HB, P, D).transpose(1, 0, 2).reshape(P, HB * D)
            )
            b1sv[s] = b1f[e].reshape(HB, P).T
            b2sv[s] = b2f[e].reshape(DB, P).T
        in_maps.append(
            {"xT": xT, "w1s": w1sv, "w2s": w2sv, "b1s": b1sv, "b2s": b2sv}
        )

    res = run_bass_kernel_spmd(
        nc, in_maps, core_ids=list(range(NCORES)), trace=_trace
    )

    out = np.zeros((T, D), dtype=np.float32)
    for core in range(NCORES):
        for s, yname in ((0, "y0"), (1, "y1")):
            e = slot_experts[core][s]
            C = CS[s]
            idx = idxs[e]
            n = len(idx)
            if n:
                yv = res.results[core][yname]
                yd = (
                    yv.reshape(P, DB, C).transpose(1, 0, 2).reshape(D, C)
                )
                out[idx] = yd[:, :n].T.astype(np.float32)
    if _trace:
        kernel.last_result = res
    return out.reshape(B, S, D)


# revision 33
# speedup vs baseline: 1.0122x; 1.0122x over previous
"""Trainium2 Bass kernel for a device-aware top-1 MoE layer.

Strategy (expert parallelism over 8 NeuronCores):
  - Host: gate + top-1 routing, then pack each expert's tokens.
    Experts are paired big+small across cores (sorted by count) so the
    program's two capacity slots (C1 >= C2) waste little padding.
  - Device (SPMD, one NEFF on 8 cores): core holds 2 experts in fp8
    e3m4 (weights pre-scaled by 2^8 / 2^9 so they sit in e3m4's normal
    range; power-of-two scales are exact).  Activations are fp16.
      stage 1: h' = relu(w1q.T @ xT + 2^8*b1)     (= 2^8 * h, exact)
      stage 2: y  = (w2q.T @ h') * 2^-17 + b2     (epilogue scale)
    fp32 PSUM accumulation throughout.
  - Host: scatter each expert's [D, count] output back to token rows.

Perf notes:
  - fp8 weights halve HBM traffic vs bf16 (8 MB/core weights) AND keep
    the PE fed: the PE consumes fp8 weights at ~250 GB/s < 358 GB/s DMA,
    so the tensor engine (not DMA) sets the pace after warmup.
  - Every DRAM tensor is host-packed into its exact SBUF image
    [128, bytes] so all DMA descriptors are long contiguous lines
    (2-16 KB), maximizing HBM efficiency.
  - All weight DMA rides the sync HWDGE queue in exact consumption
    order; stage 2 iterates h-outer so the PE consumes w2 tiles as they
    arrive.  Slot-0 output is written early on the gpsimd queue to
    overlap slot-1 compute.
"""

import numpy as np
import ml_dtypes

D = 1024
H = 2048
E = 16
NCORES = 8
P = 128
DB = D // P   # 8 d-chunks
HB = H // P   # 16 h-chunks
W1G = 2       # d-chunks per slot-1 w1 DMA (0.5 MB, 4 KB descriptors)
W2G = 4       # h-chunks per w2 DMA (0.5 MB, 4 KB descriptors)
S1 = 256.0    # 2^8  w1 scale
S2 = 512.0    # 2^9  w2 scale
UNSCALE = 1.0 / (S1 * S2)

_program_cache = {}


def _build_program(C1, C2):
    """Trace the per-core Bass/Tile program for capacities (C1, C2)."""
    import concourse.tile as tile
    from concourse import bacc, mybir

    assert C1 <= 512 and C2 <= C1
    f32 = mybir.dt.float32
    f16 = mybir.dt.float16
    f8 = mybir.dt.float8e3
    AF = mybir.ActivationFunctionType
    ALU = mybir.AluOpType
    CS = (C1, C2)

    nc = bacc.Bacc(
        "TRN2", target_bir_lowering=False, debug=False, num_devices=NCORES
    )
    xT = nc.dram_tensor("xT", [P, DB * (C1 + C2)], f16, kind="ExternalInput").ap()
    w1s = nc.dram_tensor("w1s", [2, P, DB * H], f8, kind="ExternalInput").ap()
    w2s = nc.dram_tensor("w2s", [2, P, HB * D], f8, kind="ExternalInput").ap()
    b1s = nc.dram_tensor("b1s", [2, P, HB], f32, kind="ExternalInput").ap()
    b2s = nc.dram_tensor("b2s", [2, P, DB], f32, kind="ExternalInput").ap()
    y0 = nc.dram_tensor("y0", [P, DB * C1], f16, kind="ExternalOutput").ap()
    y1 = nc.dram_tensor("y1", [P, DB * C2], f16, kind="ExternalOutput").ap()
    ys = (y0, y1)

    with tile.TileContext(nc) as tc:
        with (
            tc.tile_pool(name="xp", bufs=1) as xp,
            tc.tile_pool(name="w1p", bufs=8) as w1p,
            tc.tile_pool(name="w2p", bufs=8) as w2p,
            tc.tile_pool(name="hp", bufs=32) as hp,
            tc.tile_pool(name="bp", bufs=2) as bp,
            tc.tile_pool(name="yp", bufs=2) as yp,
            tc.tile_pool(name="ps", bufs=8, space="PSUM") as ps,
        ):
            xts = [None, None]
            hts = [[None] * HB for _ in range(2)]
            b1ts = [None, None]
            b2ts = [None, None]

            # Tiny bias tiles ride the gpsimd queue.
            for s in range(2):
                b1t = bp.tile([P, HB], f32, tag="b1")
                nc.gpsimd.dma_start(b1t[:], b1s[s])
                b1ts[s] = b1t
                b2t = bp.tile([P, DB], f32, tag="b2")
                nc.gpsimd.dma_start(b2t[:], b2s[s])
                b2ts[s] = b2t

            def epi1(i, out_t, acc_t, bias_col):
                """relu(acc + b1s) from PSUM to SBUF, alternating engines."""
                if i % 2 == 0:
                    nc.scalar.activation(out_t[:], acc_t[:], AF.Relu, bias=bias_col)
                else:
                    nc.vector.tensor_scalar(
                        out_t[:], acc_t[:], bias_col, 0.0, ALU.add, ALU.max
                    )

            def epi2(i, out_t, acc_t, bias_col):
                """acc * 2^-17 + b2 from PSUM to SBUF, alternating engines."""
                if i % 2 == 0:
                    nc.scalar.activation(
                        out_t[:], acc_t[:], AF.Identity,
                        bias=bias_col, scale=UNSCALE,
                    )
                else:
                    nc.vector.tensor_scalar(
                        out_t[:], acc_t[:], UNSCALE, bias_col, ALU.mult, ALU.add
                    )

            xoff = [0, DB * C1]

            # ---------------- slot 0 (big expert, C1) ----------------
            # Head-latency critical.  Two hardware DGEs exist (sync=SP
            # and scalar=Activation): x rides the scalar DGE while the
            # weight stream owns the sync DGE, so descriptor generation
            # for both runs in parallel and the first matmul's inputs
            # (x[d0] + w1[d0], ~0.3 MB) land as early as possible.
            C = C1
            # Everything on the sync DGE in exact consumption order: both
            # HWDGE queues share the same 16 physical DMA engines (no
            # extra bandwidth), so a single in-order stream keeps the
            # d-outer ramp starvation-free.
            xa = xp.tile([P, C], f16, tag="xa")
            nc.sync.dma_start(xa[:], xT[:, 0:C])
            # The first matmuls need only w1[d0]'s first half-H columns
            # (stage 1's half-0 pass): load d0 as two half-chunks so the
            # PE-gating transfer is 0.125 MB, not 0.25 MB.  d0's second
            # half queues after d1..d7 -- needed ~4 us later.
            w1d0h = []
            w1t = w1p.tile([P, H // 2], f8, tag="w1h", bufs=2)
            nc.sync.dma_start(w1t[:], w1s[0, :, 0:H // 2])
            w1d0h.append(w1t)
            xb = xp.tile([P, (DB - 1) * C], f16, tag="xb")
            nc.sync.dma_start(xb[:], xT[:, C:DB * C])
            w1ts0 = [None]
            for d in range(1, DB):
                w1t = w1p.tile([P, H], f8, tag="w1a")
                nc.sync.dma_start(w1t[:], w1s[0, :, d * H:(d + 1) * H])
                w1ts0.append((w1t[:], 0))
            w1t = w1p.tile([P, H // 2], f8, tag="w1h", bufs=2)
            nc.sync.dma_start(w1t[:], w1s[0, :, H // 2:H])
            w1d0h.append(w1t)

            def rhs0(d):
                return xa[:] if d == 0 else xb[:, (d - 1) * C:d * C]

            # stage 1: two half-H passes, d-outer within each, so the PE
            # starts on w1[d0] immediately (second pass reuses resident
            # w1 tiles).
            for half in range(2):
                accs1 = [
                    ps.tile([P, C], f32, tag="acc", name=f"acc1_{half}_{i}")
                    for i in range(HB // 2)
                ]
                for d in range(DB):
                    for hh in range(HB // 2):
                        h = half * (HB // 2) + hh
                        if d == 0:
                            lhs = w1d0h[half][:, hh * P:(hh + 1) * P]
                        else:
                            w1t, dd = w1ts0[d]
                            lhs = w1t[:, dd * H + h * P:dd * H + h * P + P]
                        nc.tensor.matmul(
                            accs1[hh][:],
                            lhsT=lhs,
                            rhs=rhs0(d),
                            start=(d == 0),
                            stop=(d == DB - 1),
                        )
                for hh in range(HB // 2):
                    h = half * (HB // 2) + hh
                    ht = hp.tile([P, C], f16, tag="hT")
                    epi1(h, ht, accs1[hh], b1ts[0][:, h:h + 1])
                    hts[0][h] = ht

            # stage 2: h-outer so the PE consumes each w2 tile as soon
            # as its DMA lands.
            accs = [
                ps.tile([P, C], f32, tag="acc", name=f"acc2_0_{d}")
                for d in range(DB)
            ]
            for g in range(HB // W2G):
                w2t = w2p.tile([P, W2G * D], f8, tag="w2")
                nc.sync.dma_start(
                    w2t[:], w2s[0, :, g * W2G * D:(g + 1) * W2G * D]
                )
                for hh in range(W2G):
                    h = g * W2G + hh
                    for d in range(DB):
                        nc.tensor.matmul(
                            accs[d][:],
                            lhsT=w2t[:, hh * D + d * P:hh * D + d * P + P],
                            rhs=hts[0][h][:],
                            start=(h == 0),
                            stop=(h == HB - 1),
                        )
            yt0 = yp.tile([P, DB * C], f16, tag="yt")
            for d in range(DB):
                epi2(d, yt0[:, d * C:(d + 1) * C], accs[d],
                     b2ts[0][:, d:d + 1])
            # Slot 0's output rides the (otherwise idle) gpsimd queue so
            # it overlaps slot 1 compute without blocking weight loads.
            nc.gpsimd.dma_start(ys[0], yt0[:])

            # ---------------- slot 1 (small expert, C2) ----------------
            C = CS[1]
            xt1 = xp.tile([P, DB * C], f16, tag="xc")
            nc.sync.dma_start(xt1[:], xT[:, xoff[1]:xoff[1] + DB * C])
            xts[1] = xt1
            w1ts = []
            for g in range(DB // W1G):
                w1t = w1p.tile([P, W1G * H], f8, tag="w1b", bufs=4)
                nc.sync.dma_start(
                    w1t[:], w1s[1, :, g * W1G * H:(g + 1) * W1G * H]
                )
                w1ts.append(w1t)
            # All of slot 1's w2 too: it is fully resident long before
            # stage 2 below reaches it (PE is the bottleneck by then).
            w2ts = []
            for g in range(HB // W2G):
                w2t = w2p.tile([P, W2G * D], f8, tag="w2")
                nc.sync.dma_start(
                    w2t[:], w2s[1, :, g * W2G * D:(g + 1) * W2G * D]
                )
                w2ts.append(w2t)

            # stage 1: h-outer retires each psum right away so the
            # epilogues pipeline with the next chain's matmuls.
            for h in range(HB):
                acc = ps.tile([P, C], f32, tag="acc")
                for d in range(DB):
                    w1t = w1ts[d // W1G]
                    col = (d % W1G) * H + h * P
                    nc.tensor.matmul(
                        acc[:],
                        lhsT=w1t[:, col:col + P],
                        rhs=xts[1][:, d * C:(d + 1) * C],
                        start=(d == 0),
                        stop=(d == DB - 1),
                    )
                ht = hp.tile([P, C], f16, tag="hT")
                epi1(h, ht, acc, b1ts[1][:, h:h + 1])
                hts[1][h] = ht

            # stage 2, d-blocked: each d-chunk runs its full h-chain,
            # retires its epilogue, and streams out in 2-chunk writes
            # while later chains still compute -- the tail after the
            # last matmul is one epilogue + one small DMA.
            yt1 = yp.tile([P, DB * C], f16, tag="yt")
            for d in range(DB):
                acc = ps.tile([P, C], f32, tag="acc", name=f"acc2_1_{d}")
                for h in range(HB):
                    g, hh = h // W2G, h % W2G
                    nc.tensor.matmul(
                        acc[:],
                        lhsT=w2ts[g][:, hh * D + d * P:hh * D + d * P + P],
                        rhs=hts[1][h][:],
                        start=(h == 0),
                        stop=(h == HB - 1),
                    )
                epi2(d, yt1[:, d * C:(d + 1) * C], acc, b2ts[1][:, d:d + 1])
                if d % 2 == 1:
                    lo = (d - 1) * C
                    nc.sync.dma_start(
                        ys[1][:, lo:(d + 1) * C], yt1[:, lo:(d + 1) * C]
                    )

    nc.compile()
    return nc


def kernel(x, gate_w, gate_b, w1, b1, w2, b2, _trace=False):
    from concourse.bass_utils import run_bass_kernel_spmd

    f16 = np.float16
    e3m4 = ml_dtypes.float8_e3m4

    x = np.asarray(x, dtype=np.float32)
    B, S, d_in = x.shape
    T = B * S
    xf = x.reshape(T, d_in)

    # --- routing (host side: this is the dispatch/sharding step) ---
    logits = xf @ np.asarray(gate_w, dtype=np.float32) + np.asarray(
        gate_b, dtype=np.float32
    )
    top1 = np.argmax(logits, axis=-1)
    idxs = [np.nonzero(top1 == e)[0] for e in range(E)]
    counts = np.array([len(i) for i in idxs])

    # Pair big+small experts per core: slot 0 gets the 8 largest.
    order = np.argsort(-counts, kind="stable")
    slot_experts = [
        (int(order[core]), int(order[2 * NCORES - 1 - core]))
        for core in range(NCORES)
    ]

    def cap(n):
        return min(512, max(16, (n + 3) // 4 * 4))

    C1 = cap(max(counts[e0] for e0, _ in slot_experts))
    C2 = cap(max(counts[e1] for _, e1 in slot_experts))
    assert all(counts[a] <= C1 and counts[b] <= C2 for a, b in slot_experts)

    if (C1, C2) not in _program_cache:
        _program_cache[(C1, C2)] = _build_program(C1, C2)
    nc = _program_cache[(C1, C2)]

    # Pre-quantize all expert weights into their SBUF image layouts.
    w1q = (np.asarray(w1, dtype=np.float32) * S1).astype(e3m4)   # [E, D, H]
    w2q = (np.asarray(w2, dtype=np.float32) * S2).astype(e3m4)   # [E, H, D]
    b1f = np.asarray(b1, dtype=np.float32) * S1
    b2f = np.asarray(b2, dtype=np.float32)
    xf16 = xf.astype(f16)
    CS = (C1, C2)

    in_maps = []
    for core in range(NCORES):
        xT = np.zeros((P, DB * (C1 + C2)), dtype=f16)
        w1sv = np.empty((2, P, DB * H), dtype=e3m4)
        w2sv = np.empty((2, P, HB * D), dtype=e3m4)
        b1sv = np.empty((2, P, HB), dtype=np.float32)
        b2sv = np.empty((2, P, DB), dtype=np.float32)
        xoff = (0, DB * C1)
        for s in range(2):
            e = slot_experts[core][s]
            C = CS[s]
            idx = idxs[e]
            n = len(idx)
            if n:
                # [p, dd*C + c] = x[idx[c], dd*128 + p]
                xs = xf16[idx].T.reshape(DB, P, n).transpose(1, 0, 2)
                xv = xT[:, xoff[s]:xoff[s] + DB * C].reshape(P, DB, C)
                xv[:, :, :n] = xs
            # [p, dd*H + h] = w1q[e][dd*128 + p, h]
            w1sv[s] = (
                w1q[e].reshape(DB, P, H).transpose(1, 0, 2).reshape(P, DB * H)
            )
            # [p, oo*D + d] = w2q[e][oo*128 + p, d]
            w2sv[s] = (
                w2q[e].reshape(HB, P, D).transpose(1, 0, 2).reshape(P, HB * D)
            )
            b1sv[s] = b1f[e].reshape(HB, P).T
            b2sv[s] = b2f[e].reshape(DB, P).T
        in_maps.append(
            {"xT": xT, "w1s": w1sv, "w2s": w2sv, "b1s": b1sv, "b2s": b2sv}
        )

    res = run_bass_kernel_spmd(
        nc, in_maps, core_ids=list(range(NCORES)), trace=_trace
    )

    out = np.zeros((T, D), dtype=np.float32)
    for core in range(NCORES):
        for s, yname in ((0, "y0"), (1, "y1")):
            e = slot_experts[core][s]
            C = CS[s]
            idx = idxs[e]
            n = len(idx)
            if n:
                yv = res.results[core][yname]
                yd = (
                    yv.reshape(P, DB, C).transpose(1, 0, 2).reshape(D, C)
                )
                out[idx] = yd[:, :n].T.astype(np.float32)
    if _trace:
        kernel.last_result = res
    return out.reshape(B, S, D)


# revision 34
# speedup vs baseline: 1.0718x; 1.0588x over previous
"""Trainium2 Bass kernel for a device-aware top-1 MoE layer.

Strategy (expert parallelism over 8 NeuronCores):
  - Host: gate + top-1 routing, then pack each expert's tokens.
    Experts are paired big+small across cores (sorted by count) so the
    program's two capacity slots (C1 >= C2) waste little padding.
  - Device (SPMD, one NEFF on 8 cores): core holds 2 experts in fp8
    e3m4 (weights pre-scaled by 2^8 / 2^9 so they sit in e3m4's normal
    range; power-of-two scales are exact).  Activations are fp16.
      stage 1: h' = relu(w1q.T @ xT + 2^8*b1)     (= 2^8 * h, exact)
      stage 2: y  = (w2q.T @ h') * 2^-17 + b2     (epilogue scale)
    fp32 PSUM accumulation throughout.
  - Host: scatter each expert's [D, count] output back to token rows.

Perf notes:
  - fp8 weights halve HBM traffic vs bf16 (8 MB/core weights) AND keep
    the PE fed: the PE consumes fp8 weights at ~250 GB/s < 358 GB/s DMA,
    so the tensor engine (not DMA) sets the pace after warmup.
  - Every DRAM tensor is host-packed into its exact SBUF image
    [128, bytes] so all DMA descriptors are long contiguous lines
    (2-16 KB), maximizing HBM efficiency.
  - All weight DMA rides the sync HWDGE queue in exact consumption
    order; stage 2 iterates h-outer so the PE consumes w2 tiles as they
    arrive.  Slot-0 output is written early on the gpsimd queue to
    overlap slot-1 compute.
"""

import numpy as np
import ml_dtypes

D = 1024
H = 2048
E = 16
NCORES = 8
P = 128
DB = D // P   # 8 d-chunks
HB = H // P   # 16 h-chunks
W1G = 2       # d-chunks per slot-1 w1 DMA (0.5 MB, 4 KB descriptors)
W2G = 4       # h-chunks per w2 DMA (0.5 MB, 4 KB descriptors)
S1 = 256.0    # 2^8  w1 scale
S2 = 512.0    # 2^9  w2 scale
UNSCALE = 1.0 / (S1 * S2)

_program_cache = {}


def _build_program(C1, C2):
    """Trace the per-core Bass/Tile program for capacities (C1, C2)."""
    import concourse.tile as tile
    from concourse import bacc, mybir

    assert C1 <= 512 and C2 <= C1
    f32 = mybir.dt.float32
    f16 = mybir.dt.float16
    f8 = mybir.dt.float8e3
    AF = mybir.ActivationFunctionType
    ALU = mybir.AluOpType
    CS = (C1, C2)

    nc = bacc.Bacc(
        "TRN2", target_bir_lowering=False, debug=False, num_devices=NCORES
    )
    xT = nc.dram_tensor("xT", [P, DB * (C1 + C2)], f16, kind="ExternalInput").ap()
    w1s = nc.dram_tensor("w1s", [2, P, DB * H], f8, kind="ExternalInput").ap()
    w2s = nc.dram_tensor("w2s", [2, P, HB * D], f8, kind="ExternalInput").ap()
    b1s = nc.dram_tensor("b1s", [2, P, HB], f32, kind="ExternalInput").ap()
    b2s = nc.dram_tensor("b2s", [2, P, DB], f32, kind="ExternalInput").ap()
    y0 = nc.dram_tensor("y0", [P, DB * C1], f16, kind="ExternalOutput").ap()
    y1 = nc.dram_tensor("y1", [P, DB * C2], f16, kind="ExternalOutput").ap()
    ys = (y0, y1)

    with tile.TileContext(nc) as tc:
        with (
            tc.tile_pool(name="xp", bufs=1) as xp,
            tc.tile_pool(name="w1p", bufs=8) as w1p,
            tc.tile_pool(name="w2p", bufs=8) as w2p,
            tc.tile_pool(name="hp", bufs=32) as hp,
            tc.tile_pool(name="bp", bufs=2) as bp,
            tc.tile_pool(name="yp", bufs=2) as yp,
            tc.tile_pool(name="ps", bufs=8, space="PSUM") as ps,
        ):
            xts = [None, None]
            hts = [[None] * HB for _ in range(2)]
            b1ts = [None, None]
            b2ts = [None, None]

            # Tiny bias tiles ride the gpsimd queue.
            for s in range(2):
                b1t = bp.tile([P, HB], f32, tag="b1")
                nc.gpsimd.dma_start(b1t[:], b1s[s])
                b1ts[s] = b1t
                b2t = bp.tile([P, DB], f32, tag="b2")
                nc.gpsimd.dma_start(b2t[:], b2s[s])
                b2ts[s] = b2t

            def epi1(i, out_t, acc_t, bias_col):
                """relu(acc + b1s) from PSUM to SBUF, alternating engines."""
                if i % 2 == 0:
                    nc.scalar.activation(out_t[:], acc_t[:], AF.Relu, bias=bias_col)
                else:
                    nc.vector.tensor_scalar(
                        out_t[:], acc_t[:], bias_col, 0.0, ALU.add, ALU.max
                    )

            def epi2(i, out_t, acc_t, bias_col):
                """acc * 2^-17 + b2 from PSUM to SBUF, alternating engines."""
                if i % 2 == 0:
                    nc.scalar.activation(
                        out_t[:], acc_t[:], AF.Identity,
                        bias=bias_col, scale=UNSCALE,
                    )
                else:
                    nc.vector.tensor_scalar(
                        out_t[:], acc_t[:], UNSCALE, bias_col, ALU.mult, ALU.add
                    )

            xoff = [0, DB * C1]

            # ---------------- slot 0 (big expert, C1) ----------------
            # Head-latency critical.  Two hardware DGEs exist (sync=SP
            # and scalar=Activation): x rides the scalar DGE while the
            # weight stream owns the sync DGE, so descriptor generation
            # for both runs in parallel and the first matmul's inputs
            # (x[d0] + w1[d0], ~0.3 MB) land as early as possible.
            C = C1
            # Everything on the sync DGE in exact consumption order: both
            # HWDGE queues share the same 16 physical DMA engines (no
            # extra bandwidth), so a single in-order stream keeps the
            # d-outer ramp starvation-free.
            # xa rides the scalar HWDGE: only 75 KB, so it doesn't steal
            # meaningful shared-engine bandwidth, and removing its kick
            # from the sync ring shifts every w1 chunk ~0.65 us earlier.
            xa = xp.tile([P, C], f16, tag="xa")
            nc.scalar.dma_start(xa[:], xT[:, 0:C])
            # The first matmuls need only w1[d0]'s first half-H columns
            # (stage 1's half-0 pass): load d0 as two half-chunks so the
            # PE-gating transfer is 0.125 MB, not 0.25 MB.  d0's second
            # half queues after d1..d7 -- needed ~4 us later.
            w1d0h = []
            w1t = w1p.tile([P, H // 2], f8, tag="w1h", bufs=2)
            nc.sync.dma_start(w1t[:], w1s[0, :, 0:H // 2])
            w1d0h.append(w1t)
            xb = xp.tile([P, (DB - 1) * C], f16, tag="xb")
            nc.sync.dma_start(xb[:], xT[:, C:DB * C])
            w1ts0 = [None]
            for d in range(1, DB):
                w1t = w1p.tile([P, H], f8, tag="w1a")
                nc.sync.dma_start(w1t[:], w1s[0, :, d * H:(d + 1) * H])
                w1ts0.append((w1t[:], 0))
            w1t = w1p.tile([P, H // 2], f8, tag="w1h", bufs=2)
            nc.sync.dma_start(w1t[:], w1s[0, :, H // 2:H])
            w1d0h.append(w1t)

            def rhs0(d):
                return xa[:] if d == 0 else xb[:, (d - 1) * C:d * C]

            # stage 1: two half-H passes, d-outer within each, so the PE
            # starts on w1[d0] immediately (second pass reuses resident
            # w1 tiles).
            for half in range(2):
                accs1 = [
                    ps.tile([P, C], f32, tag="acc", name=f"acc1_{half}_{i}")
                    for i in range(HB // 2)
                ]
                for d in range(DB):
                    for hh in range(HB // 2):
                        h = half * (HB // 2) + hh
                        if d == 0:
                            lhs = w1d0h[half][:, hh * P:(hh + 1) * P]
                        else:
                            w1t, dd = w1ts0[d]
                            lhs = w1t[:, dd * H + h * P:dd * H + h * P + P]
                        nc.tensor.matmul(
                            accs1[hh][:],
                            lhsT=lhs,
                            rhs=rhs0(d),
                            start=(d == 0),
                            stop=(d == DB - 1),
                        )
                for hh in range(HB // 2):
                    h = half * (HB // 2) + hh
                    ht = hp.tile([P, C], f16, tag="hT")
                    epi1(h, ht, accs1[hh], b1ts[0][:, h:h + 1])
                    hts[0][h] = ht

            # stage 2: h-outer so the PE consumes each w2 tile as soon
            # as its DMA lands.
            accs = [
                ps.tile([P, C], f32, tag="acc", name=f"acc2_0_{d}")
                for d in range(DB)
            ]
            for g in range(HB // W2G):
                w2t = w2p.tile([P, W2G * D], f8, tag="w2")
                nc.sync.dma_start(
                    w2t[:], w2s[0, :, g * W2G * D:(g + 1) * W2G * D]
                )
                for hh in range(W2G):
                    h = g * W2G + hh
                    for d in range(DB):
                        nc.tensor.matmul(
                            accs[d][:],
                            lhsT=w2t[:, hh * D + d * P:hh * D + d * P + P],
                            rhs=hts[0][h][:],
                            start=(h == 0),
                            stop=(h == HB - 1),
                        )
            yt0 = yp.tile([P, DB * C], f16, tag="yt")
            for d in range(DB):
                epi2(d, yt0[:, d * C:(d + 1) * C], accs[d],
                     b2ts[0][:, d:d + 1])
            # Slot 0's output rides the (otherwise idle) gpsimd queue so
            # it overlaps slot 1 compute without blocking weight loads.
            nc.gpsimd.dma_start(ys[0], yt0[:])

            # ---------------- slot 1 (small expert, C2) ----------------
            C = CS[1]
            xt1 = xp.tile([P, DB * C], f16, tag="xc")
            nc.sync.dma_start(xt1[:], xT[:, xoff[1]:xoff[1] + DB * C])
            xts[1] = xt1
            w1ts = []
            for g in range(DB // W1G):
                w1t = w1p.tile([P, W1G * H], f8, tag="w1b", bufs=4)
                nc.sync.dma_start(
                    w1t[:], w1s[1, :, g * W1G * H:(g + 1) * W1G * H]
                )
                w1ts.append(w1t)
            # All of slot 1's w2 too: it is fully resident long before
            # stage 2 below reaches it (PE is the bottleneck by then).
            w2ts = []
            for g in range(HB // W2G):
                w2t = w2p.tile([P, W2G * D], f8, tag="w2")
                nc.sync.dma_start(
                    w2t[:], w2s[1, :, g * W2G * D:(g + 1) * W2G * D]
                )
                w2ts.append(w2t)

            # stage 1: h-outer retires each psum right away so the
            # epilogues pipeline with the next chain's matmuls.
            for h in range(HB):
                acc = ps.tile([P, C], f32, tag="acc")
                for d in range(DB):
                    w1t = w1ts[d // W1G]
                    col = (d % W1G) * H + h * P
                    nc.tensor.matmul(
                        acc[:],
                        lhsT=w1t[:, col:col + P],
                        rhs=xts[1][:, d * C:(d + 1) * C],
                        start=(d == 0),
                        stop=(d == DB - 1),
                    )
                ht = hp.tile([P, C], f16, tag="hT")
                epi1(h, ht, acc, b1ts[1][:, h:h + 1])
                hts[1][h] = ht

            # stage 2, d-blocked: each d-chunk runs its full h-chain,
            # retires its epilogue, and streams out in 2-chunk writes
            # while later chains still compute -- the tail after the
            # last matmul is one epilogue + one small DMA.
            yt1 = yp.tile([P, DB * C], f16, tag="yt")
            for d in range(DB):
                acc = ps.tile([P, C], f32, tag="acc", name=f"acc2_1_{d}")
                for h in range(HB):
                    g, hh = h // W2G, h % W2G
                    nc.tensor.matmul(
                        acc[:],
                        lhsT=w2ts[g][:, hh * D + d * P:hh * D + d * P + P],
                        rhs=hts[1][h][:],
                        start=(h == 0),
                        stop=(h == HB - 1),
                    )
                epi2(d, yt1[:, d * C:(d + 1) * C], acc, b2ts[1][:, d:d + 1])
                if d % 2 == 1:
                    lo = (d - 1) * C
                    nc.sync.dma_start(
                        ys[1][:, lo:(d + 1) * C], yt1[:, lo:(d + 1) * C]
                    )

    nc.compile()
    return nc


def kernel(x, gate_w, gate_b, w1, b1, w2, b2, _trace=False):
    from concourse.bass_utils import run_bass_kernel_spmd

    f16 = np.float16
    e3m4 = ml_dtypes.float8_e3m4

    x = np.asarray(x, dtype=np.float32)
    B, S, d_in = x.shape
    T = B * S
    xf = x.reshape(T, d_in)

    # --- routing (host side: this is the dispatch/sharding step) ---
    logits = xf @ np.asarray(gate_w, dtype=np.float32) + np.asarray(
        gate_b, dtype=np.float32
    )
    top1 = np.argmax(logits, axis=-1)
    idxs = [np.nonzero(top1 == e)[0] for e in range(E)]
    counts = np.array([len(i) for i in idxs])

    # Pair big+small experts per core: slot 0 gets the 8 largest.
    order = np.argsort(-counts, kind="stable")
    slot_experts = [
        (int(order[core]), int(order[2 * NCORES - 1 - core]))
        for core in range(NCORES)
    ]

    def cap(n):
        return min(512, max(16, (n + 3) // 4 * 4))

    C1 = cap(max(counts[e0] for e0, _ in slot_experts))
    C2 = cap(max(counts[e1] for _, e1 in slot_experts))
    assert all(counts[a] <= C1 and counts[b] <= C2 for a, b in slot_experts)

    if (C1, C2) not in _program_cache:
        _program_cache[(C1, C2)] = _build_program(C1, C2)
    nc = _program_cache[(C1, C2)]

    # Pre-quantize all expert weights into their SBUF image layouts.
    w1q = (np.asarray(w1, dtype=np.float32) * S1).astype(e3m4)   # [E, D, H]
    w2q = (np.asarray(w2, dtype=np.float32) * S2).astype(e3m4)   # [E, H, D]
    b1f = np.asarray(b1, dtype=np.float32) * S1
    b2f = np.asarray(b2, dtype=np.float32)
    xf16 = xf.astype(f16)
    CS = (C1, C2)

    in_maps = []
    for core in range(NCORES):
        xT = np.zeros((P, DB * (C1 + C2)), dtype=f16)
        w1sv = np.empty((2, P, DB * H), dtype=e3m4)
        w2sv = np.empty((2, P, HB * D), dtype=e3m4)
        b1sv = np.empty((2, P, HB), dtype=np.float32)
        b2sv = np.empty((2, P, DB), dtype=np.float32)
        xoff = (0, DB * C1)
        for s in range(2):
            e = slot_experts[core][s]
            C = CS[s]
            idx = idxs[e]
            n = len(idx)
            if n:
                # [p, dd*C + c] = x[idx[c], dd*128 + p]
                xs = xf16[idx].T.reshape(DB, P, n).transpose(1, 0, 2)
                xv = xT[:, xoff[s]:xoff[s] + DB * C].reshape(P, DB, C)
                xv[:, :, :n] = xs
            # [p, dd*H + h] = w1q[e][dd*128 + p, h]
            w1sv[s] = (
                w1q[e].reshape(DB, P, H).transpose(1, 0, 2).reshape(P, DB * H)
            )
            # [p, oo*D + d] = w2q[e][oo*128 + p, d]
            w2sv[s] = (
                w2q[e].reshape(HB, P, D).transpose(1, 0, 2).reshape(P, HB * D)
            )
            b1sv[s] = b1f[e].reshape(HB, P).T
            b2sv[s] = b2f[e].reshape(DB, P).T
        in_maps.append(
            {"xT": xT, "w1s": w1sv, "w2s": w2sv, "b1s": b1sv, "b2s": b2sv}
        )

    res = run_bass_kernel_spmd(
        nc, in_maps, core_ids=list(range(NCORES)), trace=_trace
    )

    out = np.zeros((T, D), dtype=np.float32)
    for core in range(NCORES):
        for s, yname in ((0, "y0"), (1, "y1")):
            e = slot_experts[core][s]
            C = CS[s]
            idx = idxs[e]
            n = len(idx)
            if n:
                yv = res.results[core][yname]
                yd = (
                    yv.reshape(P, DB, C).transpose(1, 0, 2).reshape(D, C)
                )
                out[idx] = yd[:, :n].T.astype(np.float32)
    if _trace:
        kernel.last_result = res
    return out.reshape(B, S, D)
